# revision 23
# baseline (speedup 1.0000x reference)
"""SLAYER SNN forward (dense -> PSP -> spike scan, x2 layers) on 8 trn2 cores.

Sharding: data-parallel over batch (B=64 -> 8 per core); weights replicated.

Per-core pipeline:
  L1 matmul  : a1 = W1 @ x_b as two fp16 matmul passes (hi/lo split of
               1024*W1; x binary so products are exact; fp32 PSUM accum).
  PSP1       : u1 = a1 (x) srm as a matmul against a Toeplitz SRM matrix
               (fp32), after PE-transposing a1 tiles.
  Fused scan : both layers' threshold/refractory scans share one stream of
               three scalar_tensor_tensor DVE ops per timestep over a
               [128, 33] state slab (32 cols = layer-1 (b, n-tile) lanes,
               col 32 = layer-2 lanes, lagged CHUNK steps).  Refractory is
               the alpha kernel k*rho^k realized as a 2-state linear
               recurrence (G, Q-tilde) plus a lag-11 correction that
               subtracts the tail the reference truncates.
  L2 (lagged): per CHUNK of timesteps: a2 = W2 @ s1 (bf16 2-split), PE
               transpose into a sliding 126-row history, PSP2 against a
               banded SRM matrix, scaled copy into the scan's col 32.
"""
import sys
import numpy as np

sys.path.insert(0, '/opt/trn_rl_repo')

import concourse.bass as bass
import concourse.tile as tile
import concourse.mybir as mybir
from concourse import bacc, bass_utils
from concourse.bass import MemorySpace

f32 = mybir.dt.float32
f16 = mybir.dt.float16
bf16 = mybir.dt.bfloat16
AOp = mybir.AluOpType

# ---- problem constants (hardcoded; kernel.py must be self-contained) ----
B, N_IN, N_HID, N_OUT, T = 64, 2312, 512, 10, 300
NCORES = 8
BC = B // NCORES            # batches per core
KP = 2432                   # N_IN padded to 19*128
KT = KP // 128              # 19 k-tiles
NH = N_HID // 128           # 4 hidden n-tiles
THETA = 10.0
RHO = float(np.exp(-1.0))           # e^-1 (refractory tau=1)
KAPPA = float(np.float32(-2.0 * THETA * np.e))  # ref[k] = KAPPA * k * RHO^k
WSCALE = 1024.0             # fp16 split scaling for W1

DEBUG_DUMPS = False
CHUNK = 50                  # layer-2 chunk (divides 300)
LAG = 2 * CHUNK             # layer-2 slot lag (2 chunks: keeps psp2 writes
                            # strictly ahead of the correction blocks)
NSLOT = T + LAG             # 400 scan slots
SLOTA = NSLOT               # allocated slots
NCOL = 33                   # 32 layer-1 cols + 1 layer-2 col
BLK = 11                    # correction block (lag-limited)
HIST = 126                  # psp2 history rows: 76 + CHUNK


def _alpha_kernel(tau, mult=1.0, EPS=0.01):
    eps = []
    for t in np.arange(0.0, float(T), 1.0):
        v = mult * t / tau * np.exp(1.0 - t / tau)
        if abs(v) < EPS and t > tau:
            break
        eps.append(v)
    return np.asarray(eps, np.float32)


SRM = _alpha_kernel(10.0)                 # len 77
NSRM = len(SRM)                           # 77

# scan constants
C1 = float(-THETA / (KAPPA * RHO))        # s = (Qt + C1) <= ub
UB_SCALE = float(-1.0 / (KAPPA * RHO))    # ub = UB_SCALE * u
R11 = RHO ** 11
CQ = float(R11)                           # coeff on Qt[t-11]
CG = float(11.0 * R11)                    # coeff on G[t-12]
CS = float(11.0 * R11 / RHO)              # coeff on s[t-11]


def _build_consts():
    """Host-precomputed constant tensors (replicated on every core)."""
    # Toeplitz SRM matrix for PSP1: Sm[t', t] = srm[t - t'], padded to 384 rows
    Sm = np.zeros((3 * 128, T), np.float32)
    for k in range(NSRM):
        idx = np.arange(T - k)
        Sm[idx, idx + k] = SRM[k]
    # Banded SRM for PSP2: rows = history index k (t' = chunk_t0 - 76 + k)
    Sband = np.zeros((128, CHUNK), np.float32)
    for k in range(HIST):
        for j in range(CHUNK):
            idx = 76 + j - k
            if 0 <= idx < NSRM:
                Sband[k, j] = SRM[idx]
    ident = np.eye(128, dtype=np.float32)
    return Sm, Sband, ident


def _split_fp16(w):
    ws = (w * np.float32(WSCALE)).astype(np.float32)
    hi = ws.astype(np.float16)
    lo = (ws - hi.astype(np.float32)).astype(np.float16)
    return hi, lo


def _build_program():
    nc = bacc.Bacc("TRN2", target_bir_lowering=False, debug=False)

    x_d = nc.dram_tensor("x16", [BC, KP, T], f16, kind="ExternalInput")
    w1hi_d = nc.dram_tensor("w1hi", [KP, N_HID], f16, kind="ExternalInput")
    w1lo_d = nc.dram_tensor("w1lo", [KP, N_HID], f16, kind="ExternalInput")
    w2hi_d = nc.dram_tensor("w2hi", [N_HID, N_OUT], bf16, kind="ExternalInput")
    w2lo_d = nc.dram_tensor("w2lo", [N_HID, N_OUT], bf16, kind="ExternalInput")
    sm_d = nc.dram_tensor("sm", [3 * 128, T], f32, kind="ExternalInput")
    sband_d = nc.dram_tensor("sband", [128, CHUNK], f32, kind="ExternalInput")
    id_d = nc.dram_tensor("ident", [128, 128], f32, kind="ExternalInput")
    out_d = nc.dram_tensor("out", [BC, N_OUT, T], f32, kind="ExternalOutput")
    if DEBUG_DUMPS:
        ud_d = nc.dram_tensor("udump", [128, NSLOT, NCOL], f32, kind="ExternalOutput")
        sd_d = nc.dram_tensor("sdump", [128, NSLOT, NCOL], bf16, kind="ExternalOutput")
        a2_d = nc.dram_tensor("a2dump", [N_OUT, T, BC], f32, kind="ExternalOutput")

    with tile.TileContext(nc) as tc:
        with (
            tc.tile_pool(name="wts", bufs=1) as wts,
            tc.tile_pool(name="xin", bufs=2) as xin,
            tc.tile_pool(name="a1p", bufs=2) as a1p,
            tc.tile_pool(name="big", bufs=1) as big,
            tc.tile_pool(name="l2", bufs=2) as l2p,
            tc.tile_pool(name="psA", bufs=2, space=MemorySpace.PSUM) as psA,
            tc.tile_pool(name="psB", bufs=2, space=MemorySpace.PSUM) as psB,
            tc.tile_pool(name="psC", bufs=2, space=MemorySpace.PSUM) as psC,
            tc.tile_pool(name="psD", bufs=2, space=MemorySpace.PSUM) as psD,
        ):
            # ---- persistent tiles ----
            w1hi = wts.tile([128, KT, N_HID], f16)
            w1lo = wts.tile([128, KT, N_HID], f16)
            sm_sb = wts.tile([128, 3, T], f32)
            sband_sb = wts.tile([128, CHUNK], f32)
            id_sb = wts.tile([128, 128], f32)
            w2hi_sb = wts.tile([128, NH, N_OUT], bf16)
            w2lo_sb = wts.tile([128, NH, N_OUT], bf16)

            U = big.tile([128, SLOTA, NCOL], f32)      # ub (scaled -u/(k*rho) + corr)
            S = big.tile([128, SLOTA, NCOL], bf16)     # spikes
            G = big.tile([128, 3, BLK, NCOL], f32)     # G ring (lag reads)
            Q = big.tile([128, 3, BLK, NCOL], f32)     # Q-tilde ring
            # a2 with 76 zero pad columns in front (and 8 behind) so each
            # chunk's psp2 history window is a full 128-row transpose
            a2_sb = big.tile([128, 384, BC], f32)      # partition=o (10 used)
            s2f = big.tile([128, T], f32)              # fp32 staging of s2

            nc.sync.dma_start(w1hi[:], w1hi_d.ap().rearrange("(k p) m -> p k m", p=128))
            nc.sync.dma_start(w1lo[:], w1lo_d.ap().rearrange("(k p) m -> p k m", p=128))
            nc.sync.dma_start(sm_sb[:], sm_d.ap().rearrange("(k p) t -> p k t", p=128))
            nc.sync.dma_start(sband_sb[:], sband_d.ap())
            nc.sync.dma_start(id_sb[:], id_d.ap())
            nc.sync.dma_start(w2hi_sb[:], w2hi_d.ap().rearrange("(k p) m -> p k m", p=128))
            nc.sync.dma_start(w2lo_sb[:], w2lo_d.ap().rearrange("(k p) m -> p k m", p=128))

            # init whole ub slab: col 32 pre-psp2, layer-1 tail slots, unused
            # lanes all read as "far below threshold"
            nc.vector.memset(U[:], -1e30)
            nc.vector.memset(a2_sb[:], 0.0)
            nc.vector.memset(s2f[:], 0.0)

            # ---- per-batch: L1 matmul -> transpose -> PSP1 -> ub ----
            for b in range(BC):
                xb = xin.tile([128, KT, T], f16, tag="xb")
                nc.sync.dma_start(xb[:], x_d.ap()[b].rearrange("(k p) t -> p k t", p=128))

                a1 = a1p.tile([128, NH, T], f32, tag="a1")
                for nh in range(NH):
                    pa = psA.tile([128, T], f32, tag="pa")
                    for k in range(KT):
                        nc.tensor.matmul(pa[:], w1hi[:, k, nh * 128:(nh + 1) * 128],
                                         xb[:, k, :], start=(k == 0), stop=False)
                    for k in range(KT):
                        nc.tensor.matmul(pa[:], w1lo[:, k, nh * 128:(nh + 1) * 128],
                                         xb[:, k, :], start=False, stop=(k == KT - 1))
                    # unscale 1/WSCALE, PSUM -> SBUF (ScalarE)
                    nc.scalar.mul(a1[:, nh, :], pa[:], 1.0 / WSCALE)

                # transpose a1 -> a1T tiles [t' 128 x n 512]
                a1t = a1p.tile([128, 3, N_HID], f32, tag="a1t")
                nc.vector.memset(a1t[:, 2, :], 0.0)
                for kt in range(3):
                    tw = 128 if kt < 2 else T - 256
                    pt = psB.tile([128, 128], f32, tag="pt")
                    for nh in range(NH):
                        nc.tensor.transpose(pt[0:tw, 0:128],
                                            a1[:, nh, kt * 128:kt * 128 + tw],
                                            id_sb[:])
                        nc.vector.tensor_copy(a1t[0:tw, kt, nh * 128:(nh + 1) * 128],
                                              pt[0:tw, 0:128])

                # PSP1: u1[n, t] += a1T[t', n].T @ Sm[t', t]; band-pruned
                for nh in range(NH):
                    pu = psC.tile([128, T], f32, tag="pu")
                    lhs = lambda k: a1t[:, k, nh * 128:(nh + 1) * 128]
                    # kt0 full width initializes every col; later k-tiles
                    # accumulate only their nonzero band
                    nc.tensor.matmul(pu[:, 0:300], lhs(0), sm_sb[:, 0, 0:300],
                                     start=True, stop=False)
                    nc.tensor.matmul(pu[:, 128:300], lhs(1), sm_sb[:, 1, 128:300],
                                     start=False, stop=False)
                    nc.tensor.matmul(pu[:, 256:300], lhs(2), sm_sb[:, 2, 256:300],
                                     start=False, stop=True)
                    # ub = UB_SCALE * u1 into scan column b*4+nh (strided over slots)
                    nc.scalar.mul(U[:, 0:T, b * NH + nh], pu[:], UB_SCALE)

            # ---- fused scan + lagged layer-2 ----
            def rpos(i):            # ring position for slot i
                c, r = divmod(i, BLK)
                return (c % 3, r)

            nc.vector.memset(G[:, 2, BLK - 1, :], 0.0)   # G[-1]
            nc.vector.memset(Q[:, 0, 0, :], 0.0)         # Qt[0]

            a2t = None
            for blk0 in range(0, NSLOT, BLK):
                bl = min(BLK, NSLOT - blk0)
                # corrections for slots [blk0, blk0+bl) (skip first block: t<11)
                if blk0 >= BLK:
                    m = blk0
                    # ub[m..] += CQ*Qt[m-11..] + CG*G[m-12..] + CS*s[m-11..]
                    cq, rq = rpos(m - BLK)
                    assert rq == 0
                    nc.vector.scalar_tensor_tensor(
                        U[:, m:m + bl, :], Q[:, cq, 0:bl, :], CQ, U[:, m:m + bl, :],
                        AOp.mult, AOp.add)
                    nc.vector.scalar_tensor_tensor(
                        U[:, m:m + bl, :], S[:, m - BLK:m - BLK + bl, :], CS,
                        U[:, m:m + bl, :], AOp.mult, AOp.add)
                    # G lag-12: slot m-12 = last of ring chunk (m/BLK - 2)
                    cg2, rg2 = rpos(m - BLK - 1)
                    assert rg2 == BLK - 1
                    nc.vector.scalar_tensor_tensor(
                        U[:, m:m + 1, :], G[:, cg2, BLK - 1:BLK, :], CG,
                        U[:, m:m + 1, :], AOp.mult, AOp.add)
                    if bl > 1:
                        cg, _ = rpos(m - BLK)
                        nc.vector.scalar_tensor_tensor(
                            U[:, m + 1:m + bl, :], G[:, cg, 0:bl - 1, :], CG,
                            U[:, m + 1:m + bl, :], AOp.mult, AOp.add)

                for i in range(blk0, blk0 + bl):
                    ci, ri = rpos(i)
                    cp, rp = rpos(i - 1) if i > 0 else (2, BLK - 1)
                    cn, rn = rpos(i + 1)
                    # s_i = (Qt_i + C1) <= ub_i
                    nc.vector.scalar_tensor_tensor(
                        S[:, i, :], Q[:, ci, ri, :], C1, U[:, i, :],
                        AOp.add, AOp.is_le)
                    # G_i = rho*G_{i-1} + s_i
                    nc.vector.scalar_tensor_tensor(
                        G[:, ci, ri, :], G[:, cp, rp, :], RHO, S[:, i, :],
                        AOp.mult, AOp.add)
                    # Qt_{i+1} = rho*Qt_i + G_i
                    if i + 1 < NSLOT + 1:
                        nc.vector.scalar_tensor_tensor(
                            Q[:, cn, rn, :], Q[:, ci, ri, :], RHO, G[:, ci, ri, :],
                            AOp.mult, AOp.add)

                # crossed a CHUNK boundary of layer-1 scan? -> lagged layer-2
                bnd = blk0 + bl
                cb = (bnd // CHUNK) * CHUNK
                if blk0 < cb <= T:
                    c = cb // CHUNK - 1           # chunk index just finished
                    t0 = c * CHUNK
                    # a2[o, t, b] = W2 @ s1 over the chunk (bf16 2-split)
                    p2 = psD.tile([N_OUT, CHUNK, BC], f32, tag="l2ps")
                    for nh in range(NH):
                        rhs = S[:, t0:t0 + CHUNK, nh:32:NH]   # [128, t, b]
                        nc.tensor.matmul(p2[:], w2hi_sb[:, nh, :], rhs,
                                         start=(nh == 0), stop=False)
                    for nh in range(NH):
                        rhs = S[:, t0:t0 + CHUNK, nh:32:NH]
                        nc.tensor.matmul(p2[:], w2lo_sb[:, nh, :], rhs,
                                         start=False, stop=(nh == NH - 1))
                    nc.vector.tensor_copy(a2_sb[0:N_OUT, 76 + t0:76 + t0 + CHUNK, :],
                                          p2[:])

                    # history transpose: a2T[k, (b,o)], k=0..127 covers
                    # t' = t0-76 .. t0+51 (rows 126/127 hit zero Sband rows)
                    pt2 = psD.tile([128, 80], f32, tag="l2ps")
                    for b in range(BC):
                        nc.tensor.transpose(
                            pt2[:, b * N_OUT:(b + 1) * N_OUT],
                            a2_sb[0:N_OUT, t0:t0 + 128, b],
                            id_sb[0:N_OUT, 0:N_OUT])
                    a2t = l2p.tile([128, 80], f32, tag="a2t")
                    nc.vector.tensor_copy(a2t[:], pt2[:])

                    # PSP2: u2[(b,o), j] = a2T.T @ Sband  -> ub col 32 (lag CHUNK)
                    pu2 = psD.tile([80, CHUNK], f32, tag="l2ps")
                    nc.tensor.matmul(pu2[:], a2t[:, :], sband_sb[:, :],
                                     start=True, stop=True)
                    nc.scalar.mul(U[0:80, t0 + LAG:t0 + LAG + CHUNK, 32],
                                  pu2[:], UB_SCALE)

            # ---- extract s2 (col 32, slots LAG..T+LAG) -> fp32 -> DRAM ----
            nc.vector.tensor_copy(s2f[0:80, :], S[0:80, LAG:T + LAG, 32])
            nc.sync.dma_start(out_d.ap().rearrange("b o t -> (b o) t"), s2f[0:80, :])
            if DEBUG_DUMPS:
                nc.sync.dma_start(ud_d.ap(), U[:, 0:NSLOT, :])
                nc.sync.dma_start(sd_d.ap(), S[:, 0:NSLOT, :])
                nc.sync.dma_start(a2_d.ap(), a2_sb[0:N_OUT, 76:76 + T, :])

    nc.compile()
    return nc


_NC_CACHE = None
TRACE = False
LAST_EXEC_NS = None


def kernel(x, W1, W2):
    global _NC_CACHE, LAST_EXEC_NS
    x = np.ascontiguousarray(np.asarray(x, np.float32))
    W1 = np.asarray(W1, np.float32)
    W2 = np.asarray(W2, np.float32)

    # host-side input marshalling (all exact transformations)
    xp = np.zeros((B, KP, T), np.float16)
    xp[:, :N_IN, :] = x                       # binary -> fp16 exact
    w1t = np.zeros((KP, N_HID), np.float32)
    w1t[:N_IN, :] = W1.T
    w1hi, w1lo = _split_fp16(w1t)
    import ml_dtypes
    w2t = W2.T.astype(np.float32)             # [512, 10]
    w2hi = w2t.astype(ml_dtypes.bfloat16)
    w2lo = (w2t - w2hi.astype(np.float32)).astype(ml_dtypes.bfloat16)
    Sm, Sband, ident = _build_consts()

    if _NC_CACHE is None:
        _NC_CACHE = _build_program()
    nc = _NC_CACHE

    in_maps = []
    for c in range(NCORES):
        in_maps.append({
            "x16": xp[c * BC:(c + 1) * BC],
            "w1hi": w1hi, "w1lo": w1lo,
            "w2hi": w2hi, "w2lo": w2lo,
            "sm": Sm, "sband": Sband, "ident": ident,
        })

    res = bass_utils.run_bass_kernel_spmd(nc, in_maps, core_ids=list(range(NCORES)),
                                          trace=TRACE)
    LAST_EXEC_NS = res.exec_time_ns
    out = np.empty((B, N_OUT, T), np.float32)
    for c in range(NCORES):
        out[c * BC:(c + 1) * BC] = res.results[c]["out"]
    return out
